# revision 1
# baseline (speedup 1.0000x reference)
"""CSSM TinyViT block on 8 TRN2 NeuronCores.

Strategy
--------
Data-parallel over batch: B=16 -> 2 samples (2048 tokens) per core.
Per core everything is fp32; matmuls run as float32r (1 cyc/row on the PE
at N>=256, ~1e-4 relative error) via AP bitcast.

Layouts: LayerNorm stats/apply run token-major ([128 tok, 384 c], free-dim
reduction via bn_stats).  All channel-mixing matmuls run channel-major
([128 c, tok] tiles, weights stationary).  PE transposes (fp32, exact)
move between the two.  The final MLP matmul consumes the hidden
activations as the *stationary* operand, producing token-major output
directly, which saves a transpose round-trip.

Host-side preprocessing folds the LayerNorm scale/bias into the
downstream weight matrices, so the device only ever normalizes.

The scan h <- g*(h@A) + (1-g)*u runs with the sign-flipped state
h' = -h so that the t=0 state ((g-1)*ub) needs no extra negation:
h'_{t+1} = g * (A^T h'_t) + m2,  m2 = (g-1)*(u+bu),  x + h = x - h'.
"""
import json
import types

import numpy as np

import concourse.bass as bass
import concourse.mybir as mybir
from concourse.tile import TileContext
from concourse.bass_utils import run_bass_kernel_spmd

F32 = mybir.dt.float32
F32R = mybir.dt.float32r
BF16 = mybir.dt.bfloat16
AF = mybir.ActivationFunctionType
OP = mybir.AluOpType

import os
DT_MM = BF16 if os.environ.get("KERNEL_MM_DT", "f32r") == "bf16" else F32R

B, H, W, C, T = 16, 32, 32, 384, 8
HID = 4 * C
EPS = 1e-6
NCORES = 8
BSH = B // NCORES              # samples per core
NTOK = BSH * H * W             # 2048 tokens per core
GTOK = 512                     # tokens per group
NG = NTOK // GTOK              # 4 groups
TPG = GTOK // 128              # 4 token-tiles per group
KT = C // 128                  # 3 channel tiles
MH = HID // 128                # 12 hidden tiles


# ---------------------------------------------------------------- bir fix --
# This container's walrus rejects instructions whose sync-wait list exceeds
# the opcode's wait slots (an SP Drain has none free).  Move excess waits
# onto EventSemaphore instructions inserted before the instruction on the
# same engine queue; waits still happen-before, so semantics are unchanged.
_WAIT_LIMITS = {"Drain": 0}
_WAIT_DEFAULT = 1


def _fix_bir_json(bj: bytes) -> bytes:
    bir = json.loads(bj)
    counter = [0]

    def fix_blocks(blocks):
        for b in blocks:
            insts = b.get("instructions")
            if insts:
                new = []
                for inst in insts:
                    si = inst.get("sync_info")
                    waits = (si or {}).get("on_wait") or []
                    limit = _WAIT_LIMITS.get(inst.get("opcode"), _WAIT_DEFAULT)
                    if len(waits) > limit:
                        n_extra = len(waits) - limit
                        extra, keep = waits[:n_extra], waits[n_extra:]
                        for wv in extra:
                            counter[0] += 1
                            new.append({
                                "name": f"I-wfix-{counter[0]}",
                                "opcode": "EventSemaphore",
                                "engine": inst["engine"],
                                "ins": [],
                                "outs": [],
                                "sync_info": {"on_update": [], "on_wait": [wv]},
                                "debug": inst.get("debug", 0),
                            })
                        si["on_wait"] = keep
                    new.append(inst)
                b["instructions"] = new
            fix_blocks(b.get("blocks") or [])

    for fn in bir.get("functions", []):
        fix_blocks(fn.get("blocks") or [])
    return json.dumps(bir).encode()


def _patch_nc(nc):
    orig = nc.to_json_bytes

    def to_json_bytes(self):
        return _fix_bir_json(orig())

    nc.to_json_bytes = types.MethodType(to_json_bytes, nc)
    return nc


# ----------------------------------------------------------- device build --
def _r(ap):
    """Reinterpret an fp32 AP as float32r for full-rate PE matmuls."""
    return ap.bitcast(F32R)


def build_nc(repeat=1):
    nc = bass.Bass()

    x_in = nc.declare_dram_parameter("x", [NTOK, C], F32, isOutput=False)
    wu_d = nc.declare_dram_parameter("wu", [C, C], DT_MM, isOutput=False)
    wg_d = nc.declare_dram_parameter("wg", [C, C], DT_MM, isOutput=False)
    a_d = nc.declare_dram_parameter("a", [C, C], DT_MM, isOutput=False)
    w1_d = nc.declare_dram_parameter("w1", [C, HID], DT_MM, isOutput=False)
    w2_d = nc.declare_dram_parameter("w2", [HID, C], DT_MM, isOutput=False)
    bias_d = nc.declare_dram_parameter("bias", [128, 2 * KT + MH], F32,
                                       isOutput=False)
    b2_d = nc.declare_dram_parameter("b2", [1, C], DT_MM, isOutput=False)
    eye_d = nc.declare_dram_parameter("eye", [128, 128], DT_MM, isOutput=False)
    ones_d = nc.declare_dram_parameter("ones", [1, 128], DT_MM, isOutput=False)
    out_d = nc.declare_dram_parameter("out", [NTOK, C], F32, isOutput=True)

    with TileContext(nc) as tc:
        with (
            tc.tile_pool(name="wp", bufs=1) as wp,
            tc.tile_pool(name="gp", bufs=2) as gp,
            tc.tile_pool(name="hidp", bufs=1) as hidp,
            tc.tile_pool(name="hp", bufs=4) as hp,
            tc.tile_pool(name="tp", bufs=3) as tp,
            tc.tile_pool(name="sp", bufs=4) as sp,
            tc.tile_pool(name="ps", bufs=6, space="PSUM") as ps,
            tc.tile_pool(name="pst", bufs=2, space="PSUM") as pst,
        ):
            # ---- weights / constants (loaded once) ----
            wu_t = [wp.tile([128, C], DT_MM, tag=f"wu{k}", name=f"wu{k}") for k in range(KT)]
            wg_t = [wp.tile([128, C], DT_MM, tag=f"wg{k}", name=f"wg{k}") for k in range(KT)]
            a_t = [wp.tile([128, C], DT_MM, tag=f"a{k}", name=f"a{k}") for k in range(KT)]
            w1_t = [wp.tile([128, HID], DT_MM, tag=f"w1{k}", name=f"w1{k}") for k in range(KT)]
            w2_t = [wp.tile([128, C], DT_MM, tag=f"w2{k}", name=f"w2{k}") for k in range(MH)]
            bias_t = wp.tile([128, 2 * KT + MH], F32, tag="bias")
            b2_t = wp.tile([1, C], DT_MM, tag="b2")
            eye_t = wp.tile([128, 128], DT_MM, tag="eye")
            ones_t = wp.tile([1, 128], DT_MM, tag="ones")
            eps_t = wp.tile([128, 1], F32, tag="eps")
            nc.vector.memset(eps_t, EPS)
            # order matters: the SP DMA queue drains in program order, and
            # the first PE work needs eye (transposes) then wu/wg (phase B).
            nc.sync.dma_start(out=eye_t, in_=eye_d[:, :])
            nc.sync.dma_start(out=bias_t, in_=bias_d[:, :])

            def load_mid_weights():
                for k in range(KT):
                    s = slice(k * 128, (k + 1) * 128)
                    nc.sync.dma_start(out=wu_t[k], in_=wu_d[s, :])
                    nc.sync.dma_start(out=wg_t[k], in_=wg_d[s, :])
                for k in range(KT):
                    s = slice(k * 128, (k + 1) * 128)
                    nc.sync.dma_start(out=a_t[k], in_=a_d[s, :])
            def load_late_weights():
                # w1/w2 are first needed ~40us in; issuing them after the
                # first pair's x loads keeps the SP queue from delaying the
                # critical path.
                for k in range(KT):
                    s = slice(k * 128, (k + 1) * 128)
                    nc.sync.dma_start(out=w1_t[k], in_=w1_d[s, :])
                for k in range(MH):
                    nc.sync.dma_start(out=w2_t[k],
                                      in_=w2_d[k * 128:(k + 1) * 128, :])
                nc.sync.dma_start(out=b2_t, in_=b2_d[:, :])
                nc.sync.dma_start(out=ones_t, in_=ones_d[:, :])

            def phase_a(grp):
                """load + norm1 + transpose -> channel-major xn"""
                st = {}
                st["x_tm"] = x_tm = gp.tile([128, TPG, C], F32, tag="x_tm",
                                            name=f"x_tm{grp}", bufs=3)
                st["xn_cm"] = xn_cm = gp.tile([128, KT, GTOK], DT_MM,
                                              tag="xn_cm", name=f"xn_cm{grp}")
                for it in range(TPG):
                    row0 = (grp * TPG + it) * 128
                    nc.sync.dma_start(out=x_tm[:, it, :],
                                      in_=x_in[row0:row0 + 128, :])
                    mv6 = sp.tile([128, 6], F32, tag="mv6")
                    nc.vector.bn_stats(out=mv6, in_=x_tm[:, it, :])
                    mv = sp.tile([128, 2], F32, tag="mv")
                    nc.vector.bn_aggr(out=mv, in_=mv6)
                    rstd = sp.tile([128, 1], F32, tag="rstd")
                    nc.scalar.activation(out=rstd, in_=mv[:, 1:2],
                                         func=AF.Sqrt, bias=eps_t, scale=1.0)
                    nc.vector.reciprocal(out=rstd, in_=rstd)
                    xn = tp.tile([128, C], DT_MM, tag="xn", bufs=2)
                    nc.vector.tensor_scalar(out=xn, in0=x_tm[:, it, :],
                                            scalar1=mv[:, 0:1], scalar2=rstd,
                                            op0=OP.subtract, op1=OP.mult)
                    pt = pst.tile([128, KT, 128], DT_MM)
                    for c in range(KT):
                        nc.tensor.transpose(pt[:, c, :],
                                            xn[:, c * 128:(c + 1) * 128],
                                            eye_t)
                    nc.vector.tensor_copy(
                        xn_cm[:, :, it * 128:(it + 1) * 128], pt)
                return st

            def phase_b(grp, st):
                """u/g projections, gate, m2"""
                xn_cm = st["xn_cm"]
                st["g"] = g_t = gp.tile([128, KT, GTOK], F32, tag="g",
                                        name=f"g{grp}")
                st["m2"] = m2_t = gp.tile([128, KT, GTOK], DT_MM, tag="m2",
                                          name=f"m2{grp}")
                for m in range(KT):
                    msl = slice(m * 128, (m + 1) * 128)
                    psu = ps.tile([128, GTOK], F32, tag="ps")
                    psg = ps.tile([128, GTOK], F32, tag="ps")
                    for k in range(KT):
                        nc.tensor.matmul(psu, wu_t[k][:, msl], xn_cm[:, k, :],
                                         start=(k == 0), stop=(k == KT - 1))
                    for k in range(KT):
                        nc.tensor.matmul(psg, wg_t[k][:, msl], xn_cm[:, k, :],
                                         start=(k == 0), stop=(k == KT - 1))
                    th = tp.tile([128, GTOK], F32, tag="th", bufs=2)
                    nc.scalar.activation(out=th, in_=psg, func=AF.Tanh,
                                         bias=bias_t[:, KT + m:KT + m + 1],
                                         scale=0.5)
                    nc.gpsimd.tensor_scalar(out=g_t[:, m, :], in0=th,
                                            scalar1=0.5, scalar2=0.5,
                                            op0=OP.mult, op1=OP.add)
                    ub = tp.tile([128, GTOK], F32, tag="ub", bufs=2)
                    nc.vector.tensor_scalar(out=ub, in0=psu,
                                            scalar1=bias_t[:, m:m + 1],
                                            scalar2=None, op0=OP.add)
                    # m2 = (g - 1) * (u + bu)   (= -(1-g)*u_b)
                    nc.vector.scalar_tensor_tensor(
                        out=m2_t[:, m, :], in0=g_t[:, m, :], scalar=1.0,
                        in1=ub, op0=OP.subtract, op1=OP.mult)
                st["h"] = m2_t

            def scan_step(grp, st):
                g_t, m2_t, h_prev = st["g"], st["m2"], st["h"]
                h_next = hp.tile([128, KT, GTOK], DT_MM, tag="h",
                                 name=f"h{grp}")
                for m in range(KT):
                    msl = slice(m * 128, (m + 1) * 128)
                    psh = ps.tile([128, GTOK], F32, tag="ps")
                    for k in range(KT):
                        nc.tensor.matmul(psh, a_t[k][:, msl], h_prev[:, k, :],
                                         start=(k == 0), stop=(k == KT - 1))
                    t1 = tp.tile([128, GTOK], F32, tag="t1")
                    nc.vector.tensor_mul(out=t1, in0=g_t[:, m, :], in1=psh)
                    eng = nc.vector if m == 2 else nc.gpsimd
                    eng.tensor_add(out=h_next[:, m, :], in0=t1,
                                   in1=m2_t[:, m, :])
                st["h"] = h_next

            def residual1(grp, st):
                """x2 = x + h = x - h'  (token-major)"""
                h_prev, x_tm = st["h"], st["x_tm"]
                st["x2_tm"] = x2_tm = gp.tile([128, TPG, C], F32, tag="x2_tm",
                                              name=f"x2_tm{grp}")
                h_tm = gp.tile([128, TPG, C], F32, tag="h_tm",
                               name=f"h_tm{grp}")
                for it in range(TPG):
                    pt = pst.tile([128, KT, 128], DT_MM)
                    for c in range(KT):
                        nc.tensor.transpose(
                            pt[:, c, :],
                            h_prev[:, c, it * 128:(it + 1) * 128], eye_t)
                    nc.vector.tensor_copy(
                        h_tm[:, it, :].rearrange("p (c q) -> p c q", c=KT),
                        pt)
                    nc.gpsimd.tensor_sub(out=x2_tm[:, it, :],
                                         in0=x_tm[:, it, :],
                                         in1=h_tm[:, it, :])

            def norm2(grp, st):
                x2_tm = st["x2_tm"]
                st["xn2_cm"] = xn2_cm = gp.tile([128, KT, GTOK], DT_MM,
                                                tag="xn2_cm",
                                                name=f"xn2_cm{grp}")
                for it in range(TPG):
                    mv6 = sp.tile([128, 6], F32, tag="mv6")
                    nc.vector.bn_stats(out=mv6, in_=x2_tm[:, it, :])
                    mv = sp.tile([128, 2], F32, tag="mv")
                    nc.vector.bn_aggr(out=mv, in_=mv6)
                    rstd = sp.tile([128, 1], F32, tag="rstd")
                    nc.scalar.activation(out=rstd, in_=mv[:, 1:2],
                                         func=AF.Sqrt, bias=eps_t, scale=1.0)
                    nc.vector.reciprocal(out=rstd, in_=rstd)
                    xn2 = tp.tile([128, C], DT_MM, tag="xn", bufs=2)
                    nc.vector.tensor_scalar(out=xn2, in0=x2_tm[:, it, :],
                                            scalar1=mv[:, 0:1], scalar2=rstd,
                                            op0=OP.subtract, op1=OP.mult)
                    pt = pst.tile([128, KT, 128], DT_MM)
                    for c in range(KT):
                        nc.tensor.transpose(pt[:, c, :],
                                            xn2[:, c * 128:(c + 1) * 128],
                                            eye_t)
                    nc.vector.tensor_copy(
                        xn2_cm[:, :, it * 128:(it + 1) * 128], pt)

            def mlp(grp, st):
                xn2_cm, x2_tm = st["xn2_cm"], st["x2_tm"]
                hid_t = hidp.tile([128, MH, GTOK], DT_MM, tag="hid",
                                  name=f"hid{grp}")
                for mh in range(MH):
                    msl = slice(mh * 128, (mh + 1) * 128)
                    psh = ps.tile([128, GTOK], F32, tag="ps")
                    for k in range(KT):
                        nc.tensor.matmul(psh, w1_t[k][:, msl],
                                         xn2_cm[:, k, :],
                                         start=(k == 0), stop=(k == KT - 1))
                    nc.scalar.activation(
                        out=hid_t[:, mh, :], in_=psh,
                        func=AF.Gelu_apprx_tanh,
                        bias=bias_t[:, 2 * KT + mh:2 * KT + mh + 1],
                        scale=1.0)
                # second matmul: hidden is the stationary operand -> output
                # lands token-major; fold mlp_b2 in via a K=1 matmul.
                for it in range(TPG):
                    tsl = slice(it * 128, (it + 1) * 128)
                    pso = ps.tile([128, C], F32, tag="ps")
                    for mh in range(MH):
                        nc.tensor.matmul(pso, hid_t[:, mh, tsl], w2_t[mh],
                                         start=(mh == 0), stop=False)
                    nc.tensor.matmul(pso, ones_t, b2_t,
                                     start=False, stop=True)
                    nc.vector.tensor_add(out=x2_tm[:, it, :],
                                         in0=x2_tm[:, it, :], in1=pso)
                    row0 = (grp * TPG + it) * 128
                    nc.sync.dma_start(out=out_d[row0:row0 + 128, :],
                                      in_=x2_tm[:, it, :])

            # Pairwise interleave groups so the PE fills one group's
            # scan/norm dependency stalls with the other group's matmuls;
            # additionally pipeline the next pair's phase A into the
            # current pair's norm2/MLP window.
            npair = (NG // 2) * repeat
            states = {}
            for pair_i in range(npair):
                pair = pair_i % (NG // 2)
                g0, g1 = 2 * pair, 2 * pair + 1
                if pair_i == 0:
                    states[g0] = phase_a(g0)
                    states[g1] = phase_a(g1)
                    load_mid_weights()
                s0, s1 = states[g0], states[g1]
                phase_b(g0, s0)
                phase_b(g1, s1)
                if pair_i == 0:
                    load_late_weights()
                for t in range(T - 1):
                    scan_step(g0, s0)
                    scan_step(g1, s1)
                residual1(g0, s0)
                residual1(g1, s1)
                norm2(g0, s0)
                norm2(g1, s1)
                if pair_i + 1 < npair:
                    nx = 2 * ((pair_i + 1) % (NG // 2))
                    states[nx] = phase_a(nx)
                    states[nx + 1] = phase_a(nx + 1)
                mlp(g0, s0)
                mlp(g1, s1)
    return nc



_NC_CACHE = {}


def _get_nc():
    if "nc" not in _NC_CACHE:
        _NC_CACHE["nc"] = _patch_nc(build_nc())
    return _NC_CACHE["nc"]


# ---------------------------------------------------------------- kernel --
def kernel(x, norm1_scale, norm1_bias, Wu, bu, Wg, bg, A,
           norm2_scale, norm2_bias, mlp_w1, mlp_b1, mlp_w2, mlp_b2,
           _return_raw=False):
    f = np.float32
    x = np.asarray(x, f)
    norm1_scale = np.asarray(norm1_scale, f)
    norm1_bias = np.asarray(norm1_bias, f)
    Wu, bu = np.asarray(Wu, f), np.asarray(bu, f)
    Wg, bg = np.asarray(Wg, f), np.asarray(bg, f)
    A = np.asarray(A, f)
    norm2_scale = np.asarray(norm2_scale, f)
    norm2_bias = np.asarray(norm2_bias, f)
    mlp_w1, mlp_b1 = np.asarray(mlp_w1, f), np.asarray(mlp_b1, f)
    mlp_w2, mlp_b2 = np.asarray(mlp_w2, f), np.asarray(mlp_b2, f)

    # fold LN affine into downstream weights
    wu = np.ascontiguousarray(norm1_scale[:, None] * Wu)
    bu_f = bu + norm1_bias @ Wu
    wg = np.ascontiguousarray(norm1_scale[:, None] * Wg)
    bg_f = bg + norm1_bias @ Wg
    w1 = np.ascontiguousarray(norm2_scale[:, None] * mlp_w1)
    b1_f = mlp_b1 + norm2_bias @ mlp_w1

    # bias pack: [128, KT + KT + MH] columns = bu tiles, bg tiles, b1 tiles
    bias = np.empty((128, 2 * KT + MH), f)
    for m in range(KT):
        bias[:, m] = bu_f[m * 128:(m + 1) * 128]
        bias[:, KT + m] = 0.5 * bg_f[m * 128:(m + 1) * 128]
    for m in range(MH):
        bias[:, 2 * KT + m] = b1_f[m * 128:(m + 1) * 128]

    eye = np.eye(128, dtype=f)
    b2row = np.ascontiguousarray(mlp_b2[None, :])
    if DT_MM == BF16:
        import ml_dtypes
        bf = ml_dtypes.bfloat16
        wu, wg, w1 = wu.astype(bf), wg.astype(bf), w1.astype(bf)
        A = A.astype(bf)
        mlp_w2 = np.asarray(mlp_w2, f).astype(bf)
        b2row = b2row.astype(bf)
        eye = eye.astype(bf)

    xs = x.reshape(NCORES, NTOK, C)
    in_maps = [{
        "x": np.ascontiguousarray(xs[i]),
        "wu": wu, "wg": wg, "a": A, "w1": w1, "w2": mlp_w2,
        "bias": bias, "b2": b2row, "eye": eye,
        "ones": np.ones((1, 128), f) if DT_MM != BF16
        else np.ones((1, 128), f).astype(__import__("ml_dtypes").bfloat16),
    } for i in range(NCORES)]

    res = run_bass_kernel_spmd(_get_nc(), in_maps, list(range(NCORES)))
    if _return_raw:
        return res
    out = np.concatenate([res.results[i]["out"] for i in range(NCORES)], axis=0)
    return out.reshape(B, H, W, C).astype(np.float32)

